# revision 21
# baseline (speedup 1.0000x reference)
"""Trainium2 Bass kernel for a GNN message-passing layer.

Math (matches the reference):
  msg_fwd(e)  = concat(H[head], E[e], H[head]+E[e], H[head]*E[e]) @ W_fwd.T + b_fwd
  msg_back(e) = concat(H[tail], E[e], H[tail]+E[e], H[tail]*E[e]) @ W_back.T + b_back
  agg[v] = mean of messages destined to v   (fwd -> tail, back -> head)
  out = LN(leaky_relu(agg) + H) * gamma + beta

Using linearity of the concat GEMM:
  msg = Hh @ (W1+W3).T + E @ (W2+W3).T + (Hh*E) @ W4.T  (+ bias)
and linearity of the segment-sum, each destination node only needs the three
768-wide raw sums  [sum Hh | sum E | sum Hh*E]  per direction, followed by a
small per-node GEMM with the combined weights.

v2: destinations (nodes) are packed into 128-node windows (50 per core). The
segment-sum is a one-hot scatter matmul; the bulk of the [Hh|E] stream is
shipped in fp8e4 and scattered with DoubleRow fp8 matmuls (2 contraction
rows per column step). Messages whose destination has a small total degree
(the nodes most sensitive to quantization error) are routed through a
full-fp16 tile per window/direction. The Hh*E products are computed on
device (DVE/GpSimd) in fp16 from the shipped operands and scattered with
plain matmuls. The per-node mean (1/cnt) rides the PSUM-drain scale; the
LayerNorm tail is balanced across Scalar/Vector/GpSimd engines.
"""

import os
import numpy as np
import ml_dtypes

import concourse.bass as bass
import concourse.bacc as bacc
import concourse.mybir as mybir
import concourse.tile as tile
from concourse.masks import make_identity
from concourse.bass_utils import run_bass_kernel_spmd

N_NODES = 50000
N_EDGES = 250000
D = 256
LEAKY = 0.01
LN_EPS = 1e-5

N_CORES = 8
WPC = 50                      # windows per core
NWIN = N_CORES * WPC          # 400 windows of <=128 nodes
CTH = 3                       # dst total-degree threshold for fp16 routing
PROFILE = bool(int(os.environ.get("KERNEL_TRACE", "0")))
LAST = {}                     # debug/profiling info from the last call

F32 = mybir.dt.float32
F16 = mybir.dt.float16
F8 = mybir.dt.float8e4
DRMODE = mybir.MatmulPerfMode.DoubleRow
E4M3 = ml_dtypes.float8_e4m3


# ----------------------------------------------------------------- host side

def _pack_nodes(cnt_f, cnt_b, cap):
    """Assign each node to one of NWIN windows (<=128 nodes each) such that
    per-window fwd/back message counts stay <= cap. Greedy min-max."""
    order = np.argsort(-(cnt_f + cnt_b), kind="stable")
    F = np.zeros(NWIN, dtype=np.int64)
    B = np.zeros(NWIN, dtype=np.int64)
    NN = np.zeros(NWIN, dtype=np.int64)
    win_of = np.empty(N_NODES, dtype=np.int64)
    loc_of = np.empty(N_NODES, dtype=np.int64)
    BIG = np.int64(1) << 60
    for v in order:
        cf = cnt_f[v]
        cb = cnt_b[v]
        score = np.maximum(F + cf, B + cb)
        bad = (NN >= 128) | (F + cf > cap) | (B + cb > cap)
        score = np.where(bad, BIG, score)
        w = int(np.argmin(score))
        if score[w] >= BIG:
            return None
        win_of[v] = w
        loc_of[v] = NN[w]
        F[w] += cf
        B[w] += cb
        NN[w] += 1
    return win_of, loc_of, NN


def _pack_host(H, E, ht, T_dr):
    capq = T_dr * 256             # fp8 (DoubleRow) slots per window-dir
    cap = capq + 128              # plus the fp16 tile
    nq = 2 * T_dr                 # fp8 128-row planes per window-dir
    heads = ht[:, 0].astype(np.int64)
    tails = ht[:, 1].astype(np.int64)
    cnt_f = np.bincount(tails, minlength=N_NODES)
    cnt_b = np.bincount(heads, minlength=N_NODES)
    cnt = cnt_f + cnt_b

    packed = _pack_nodes(cnt_f, cnt_b, cap)
    if packed is None:
        return None
    win_of, loc_of, NN = packed

    H8 = H.astype(E4M3)
    E8 = E.astype(E4M3)
    H16 = H.astype(np.float16)
    E16 = E.astype(np.float16)

    # partition-major layouts so each DMA is one contiguous run per partition
    stream8 = np.zeros((NWIN, 2, 128, nq, 2 * D), dtype=E4M3)
    stream16 = np.zeros((NWIN, 2, 128, 2 * D), dtype=np.float16)
    ind8 = np.zeros((NWIN, 2, 128, nq, 128), dtype=E4M3)
    ind16 = np.zeros((NWIN, 2, 128, 128), dtype=np.float16)

    for d, (src, dst) in enumerate(((heads, tails), (tails, heads))):
        w = win_of[dst]
        c = cnt[dst]
        order = np.lexsort((c, w))        # window-major, low total degree first
        ws = w[order]
        counts = np.bincount(ws, minlength=NWIN)
        starts = np.zeros(NWIN + 1, dtype=np.int64)
        np.cumsum(counts, out=starts[1:])
        pos = np.arange(len(ws), dtype=np.int64) - starts[ws]
        lowc = np.bincount(ws[c[order] <= CTH], minlength=NWIN)
        k16 = np.minimum(128, np.minimum(
            counts, np.maximum(counts - capq, lowc)))
        is16 = pos < k16[ws]

        e16 = order[is16]
        s16 = pos[is16]
        stream16[ws[is16], d, s16, :D] = H16[src[e16]]
        stream16[ws[is16], d, s16, D:] = E16[e16]
        ind16[ws[is16], d, s16, loc_of[dst[e16]]] = 1.0

        e8 = order[~is16]
        r8 = pos[~is16] - k16[ws[~is16]]
        q8 = r8 >> 7                      # plane index
        p8 = r8 & 127                     # partition
        w8 = ws[~is16]
        stream8[w8, d, p8, q8, :D] = H8[src[e8]]
        stream8[w8, d, p8, q8, D:] = E8[e8]
        ind8[w8, d, p8, q8, loc_of[dst[e8]]] = 1.0

    node_ids = np.full((NWIN, 128), -1, dtype=np.int64)
    node_ids[win_of, loc_of] = np.arange(N_NODES, dtype=np.int64)

    recip_all = 1.0 / np.maximum(cnt, 1).astype(np.float32)
    safe_ids = np.maximum(node_ids, 0)
    hres = H16[safe_ids]                     # [NWIN, 128, D]
    hres[node_ids < 0] = 0.0
    recip = recip_all[safe_ids].astype(np.float32)
    recip[node_ids < 0] = 1.0

    # batch pairs of windows into single contiguous-per-partition DMA images
    npr = WPC // 2
    s8 = stream8.reshape(N_CORES, npr, 2, 2, 128, nq, 2 * D)
    s8 = np.ascontiguousarray(s8.transpose(0, 1, 4, 2, 3, 5, 6)).reshape(
        N_CORES, npr, 128, 4 * nq, 2 * D)
    s16 = stream16.reshape(N_CORES, npr, 2, 2, 128, 2 * D)
    s16 = np.ascontiguousarray(s16.transpose(0, 1, 4, 2, 3, 5)).reshape(
        N_CORES, npr, 128, 4, 2 * D)
    i8 = ind8.reshape(N_CORES, npr, 2, 2, 128, nq, 128)
    i8 = np.ascontiguousarray(i8.transpose(0, 1, 4, 2, 3, 5, 6)).reshape(
        N_CORES, npr, 128, 4 * nq, 128)
    i16 = ind16.reshape(N_CORES, npr, 2, 2, 128, 128)
    i16 = np.ascontiguousarray(i16.transpose(0, 1, 4, 2, 3, 5)).reshape(
        N_CORES, npr, 128, 4, 128)
    hr = hres.reshape(N_CORES, npr, 2, 128, D)
    hr = np.ascontiguousarray(hr.transpose(0, 1, 3, 2, 4))
    return {
        "stream8": s8,
        "stream16": s16,
        "ind8": i8,
        "ind16": i16,
        "hres": hr,
        "recip": recip.reshape(N_CORES, WPC, 128).transpose(0, 2, 1).copy(),
        "node_ids": node_ids,
        "cnt_f": cnt_f,
        "cnt_b": cnt_b,
        "cnt": cnt,
    }


def _weights_pack(W_fwd, W_back):
    def cat(W):
        W1, W2, W3, W4 = (W[:, i * D:(i + 1) * D] for i in range(4))
        return np.concatenate([(W1 + W3).T, (W2 + W3).T, W4.T], axis=0)

    wf = np.ascontiguousarray(cat(W_fwd).reshape(6, 128, D), dtype=np.float16)
    wb6 = cat(W_back).reshape(6, 128, D)
    # acc layout: blocks 0..3 f[Hh|E], 4..5 f[HE], 6..7 b[HE], 8..11 b[Hh|E]
    wb = np.ascontiguousarray(wb6[[4, 5, 0, 1, 2, 3]], dtype=np.float16)
    return wf, wb


# --------------------------------------------------------------- device side

def _build_nc(T_dr, use_bias, use_gb):
    nc = bacc.Bacc()
    nq = 2 * T_dr
    npr = WPC // 2

    st8_d = nc.dram_tensor("stream8", [npr, 128, 4 * nq, 2 * D], F8,
                           kind="ExternalInput")
    st16_d = nc.dram_tensor("stream16", [npr, 128, 4, 2 * D], F16,
                            kind="ExternalInput")
    ind8_d = nc.dram_tensor("ind8", [npr, 128, 4 * nq, 128], F8,
                            kind="ExternalInput")
    ind16_d = nc.dram_tensor("ind16", [npr, 128, 4, 128], F16,
                             kind="ExternalInput")
    hres_d = nc.dram_tensor("hres", [npr, 128, 2, D], F16,
                            kind="ExternalInput")
    recip_d = nc.dram_tensor("recip", [128, WPC], F32, kind="ExternalInput")
    wf_d = nc.dram_tensor("wf", [6, 128, D], F16, kind="ExternalInput")
    wb_d = nc.dram_tensor("wb", [6, 128, D], F16, kind="ExternalInput")
    if use_bias:
        bc_d = nc.dram_tensor("bc", [WPC * 128, D], F32, kind="ExternalInput")
    if use_gb:
        gam_d = nc.dram_tensor("gam", [1, D], F32, kind="ExternalInput")
        bet_d = nc.dram_tensor("bet", [1, D], F32, kind="ExternalInput")
    out_d = nc.dram_tensor("out", [npr, 128, 2, D], F16,
                           kind="ExternalOutput")

    with tile.TileContext(nc) as tc:
        with (
            tc.tile_pool(name="const", bufs=1) as constp,
            tc.tile_pool(name="stream", bufs=3) as streamp,
            tc.tile_pool(name="he", bufs=8) as hep,
            tc.tile_pool(name="ind", bufs=3) as indp,
            tc.tile_pool(name="aggsb", bufs=3) as aggsbp,
            tc.tile_pool(name="aggT", bufs=3) as aggTp,
            tc.tile_pool(name="tailp", bufs=3) as tailp,
            tc.tile_pool(name="outp", bufs=4) as outp,
            tc.tile_pool(name="pacc", bufs=2, space="PSUM") as pacc,
            tc.tile_pool(name="pmisc", bufs=1, space="PSUM") as pmisc,
        ):
            ident32 = constp.tile([128, 128], F32)
            make_identity(nc, ident32)
            ident = constp.tile([128, 128], F16)
            nc.vector.tensor_copy(out=ident, in_=ident32)
            wf_sb = constp.tile([128, 6, D], F16)
            nc.sync.dma_start(out=wf_sb,
                              in_=wf_d[:, :, :].rearrange("c k n -> k c n"))
            wb_sb = constp.tile([128, 6, D], F16)
            nc.sync.dma_start(out=wb_sb,
                              in_=wb_d[:, :, :].rearrange("c k n -> k c n"))
            recip_sb = constp.tile([128, WPC], F32)
            nc.sync.dma_start(out=recip_sb, in_=recip_d[:, :])
            eps_sb = constp.tile([128, 1], F32)
            nc.vector.memset(eps_sb, LN_EPS)
            if use_gb:
                gam_sb = constp.tile([128, D], F32)
                nc.sync.dma_start(
                    out=gam_sb,
                    in_=bass.AP(tensor=gam_d, offset=0, ap=[[0, 128], [1, D]]),
                )
                bet_sb = constp.tile([128, D], F32)
                nc.sync.dma_start(
                    out=bet_sb,
                    in_=bass.AP(tensor=bet_d, offset=0, ap=[[0, 128], [1, D]]),
                )

            def node_side(w, acc, hresp, op):
                """Drain, transpose, per-node GEMM and LayerNorm tail for
                window w (emitted one window behind the scatter so the
                drain/transpose ping-pong hides behind scatter matmuls)."""
                wlo = w & 1
                # drain PSUM to fp16 in halves (raw sums; 1/cnt at prelu)
                aggsb = aggsbp.tile([128, 1536], F16)
                nc.scalar.copy(out=aggsb[:, 0:768], in_=acc[:, 0:768])
                nc.scalar.copy(out=aggsb[:, 768:1536], in_=acc[:, 768:1536])

                aggTh = []
                for h in range(2):
                    tp = pmisc.tile([128, 768], F16, tag="tp")
                    for j in range(6):
                        blk = h * 6 + j
                        nc.tensor.transpose(
                            tp[:, j * 128:(j + 1) * 128],
                            aggsb[:, blk * 128:(blk + 1) * 128], ident)
                    aggT = aggTp.tile([128, 6, 128], F16, tag=f"t{h}")
                    if h == 0:
                        nc.vector.tensor_copy(out=aggT, in_=tp)
                    else:
                        nc.scalar.copy(out=aggT, in_=tp)
                    aggTh.append(aggT)

                nodeps = pmisc.tile([128, D], F32, tag="nd")
                for blk in range(12):
                    rhs = (wf_sb[:, blk, :] if blk < 6
                           else wb_sb[:, blk - 6, :])
                    nc.tensor.matmul(
                        nodeps, aggTh[blk // 6][:, blk % 6, :], rhs,
                        start=(blk == 0), stop=(blk == 11))

                x = tailp.tile([128, D], F32, tag="x")
                if use_bias:
                    y = tailp.tile([128, D], F32, tag="y")
                    nc.scalar.activation(
                        out=y, in_=nodeps,
                        func=mybir.ActivationFunctionType.Copy,
                        bias=0.0, scale=recip_sb[:, w:w + 1])
                    bc_sb = tailp.tile([128, D], F32, tag="bc")
                    nc.sync.dma_start(
                        out=bc_sb, in_=bc_d[w * 128:(w + 1) * 128, :])
                    nc.gpsimd.tensor_add(y, y, bc_sb)
                    nc.scalar.activation(
                        out=x, in_=y,
                        func=mybir.ActivationFunctionType.Prelu,
                        bias=0.0, scale=1.0, alpha=LEAKY)
                else:
                    nc.scalar.activation(
                        out=x, in_=nodeps,
                        func=mybir.ActivationFunctionType.Prelu,
                        bias=0.0, scale=recip_sb[:, w:w + 1], alpha=LEAKY)

                nc.gpsimd.tensor_add(x, x, hresp[:, wlo, :])

                stats = tailp.tile([128, 6], F32, tag="stats")
                nc.vector.bn_stats(out=stats, in_=x)
                mv = tailp.tile([128, 2], F32, tag="mv")
                nc.vector.bn_aggr(out=mv, in_=stats)
                std = tailp.tile([128, 1], F32, tag="std")
                nc.scalar.activation(
                    out=std, in_=mv[:, 1:2],
                    func=mybir.ActivationFunctionType.Sqrt,
                    bias=eps_sb, scale=1.0)
                rstd = tailp.tile([128, 1], F32, tag="rstd")
                nc.vector.reciprocal(out=rstd, in_=std)
                nmr = tailp.tile([128, 1], F32, tag="nmr")
                nc.vector.tensor_scalar(
                    out=nmr, in0=mv[:, 0:1], scalar1=rstd, scalar2=-1.0,
                    op0=mybir.AluOpType.mult, op1=mybir.AluOpType.mult)

                nc.scalar.activation(
                    out=op[:, wlo, :], in_=x,
                    func=mybir.ActivationFunctionType.Identity,
                    bias=nmr, scale=rstd)
                if use_gb:
                    nc.vector.tensor_tensor(
                        out=op[:, wlo, :], in0=op[:, wlo, :], in1=gam_sb,
                        op=mybir.AluOpType.mult)
                    nc.vector.tensor_tensor(
                        out=op[:, wlo, :], in0=op[:, wlo, :], in1=bet_sb,
                        op=mybir.AluOpType.add)
                if wlo == 1:
                    # store from the gpsimd queue: the sync queue is in-order
                    # and a store waiting on compute would block the next
                    # pair's input prefetch
                    nc.gpsimd.dma_start(out=out_d[w // 2], in_=op)

            pending = None  # (w, acc, hresp, op) awaiting node_side
            for pr in range(npr):
                st8p = streamp.tile([128, 4 * nq, 2 * D], F8, tag="st8")
                nc.sync.dma_start(out=st8p, in_=st8_d[pr])
                st16p = streamp.tile([128, 4, 2 * D], F16, tag="st16")
                nc.sync.dma_start(out=st16p, in_=st16_d[pr])
                ind8p = indp.tile([128, 4 * nq, 128], F8, tag="i8")
                nc.sync.dma_start(out=ind8p, in_=ind8_d[pr])
                ind16p = indp.tile([128, 4, 128], F16, tag="i16")
                nc.sync.dma_start(out=ind16p, in_=ind16_d[pr])
                hresp = tailp.tile([128, 2, D], F16, tag="hres")
                nc.sync.dma_start(out=hresp, in_=hres_d[pr])
                op = outp.tile([128, 2, D], F16)

                # products for all four window-dir groups up front
                he8s = []
                he16s = []
                for g in range(4):
                    base = g * nq
                    he8 = hep.tile([128, nq, D], F16, tag="he8")
                    nc.vector.tensor_tensor(
                        out=he8, in0=st8p[:, base:base + nq, 0:D],
                        in1=st8p[:, base:base + nq, D:2 * D],
                        op=mybir.AluOpType.mult)
                    he8s.append(he8)
                    he16 = hep.tile([128, D], F16, tag="he16")
                    nc.gpsimd.tensor_tensor(
                        out=he16, in0=st16p[:, g, 0:D],
                        in1=st16p[:, g, D:2 * D],
                        op=mybir.AluOpType.mult)
                    he16s.append(he16)

                for wlo in range(2):
                    w = pr * 2 + wlo
                    # acc columns (fp32, 3 PSUM banks):
                    #   0:512    f [sumHh | sumE]
                    #   512:768  f [sumHE]
                    #   768:1024 b [sumHE]
                    #   1024:1536 b [sumHh | sumE]
                    acc = pacc.tile([128, 1536], F32)
                    # [Hh|E] for both directions first (only needs the DMAs),
                    # then the HE matmuls (need the DVE/GpSimd products)
                    for d in range(2):
                        g = wlo * 2 + d
                        base = g * nq
                        cs = (0, 512) if d == 0 else (1024, 1536)
                        for t in range(T_dr):
                            nc.tensor.matmul(
                                acc[:, cs[0]:cs[1]],
                                ind8p[:, base + 2 * t:base + 2 * t + 2, :],
                                st8p[:, base + 2 * t:base + 2 * t + 2, :],
                                start=(t == 0), stop=False, perf_mode=DRMODE)
                        nc.tensor.matmul(
                            acc[:, cs[0]:cs[1]], ind16p[:, g, :],
                            st16p[:, g, :], start=False, stop=True)
                    for d in range(2):
                        g = wlo * 2 + d
                        base = g * nq
                        ch = (512, 768) if d == 0 else (768, 1024)
                        for q in range(nq):
                            nc.tensor.matmul(
                                acc[:, ch[0]:ch[1]],
                                ind8p[:, base + q, :], he8s[g][:, q, :],
                                start=(q == 0), stop=False)
                        nc.tensor.matmul(
                            acc[:, ch[0]:ch[1]], ind16p[:, g, :], he16s[g],
                            start=False, stop=True)

                    if pending is not None:
                        node_side(*pending)
                    pending = (w, acc, hresp, op)
            node_side(*pending)

    nc.compile()
    return nc


_NC_CACHE = {}


def kernel(H, E, ht, W_fwd, b_fwd, W_back, b_back, gamma, beta):
    H = np.asarray(H, dtype=np.float32)
    E = np.asarray(E, dtype=np.float32)
    ht = np.asarray(ht)
    W_fwd = np.asarray(W_fwd, dtype=np.float32)
    W_back = np.asarray(W_back, dtype=np.float32)
    b_fwd = np.asarray(b_fwd, dtype=np.float32)
    b_back = np.asarray(b_back, dtype=np.float32)
    gamma = np.asarray(gamma, dtype=np.float32)
    beta = np.asarray(beta, dtype=np.float32)

    T_dr = 2
    pk = _pack_host(H, E, ht, T_dr)
    if pk is None:
        T_dr = 3
        pk = _pack_host(H, E, ht, T_dr)
        assert pk is not None, "window packing failed even at T_dr=3"

    wf, wb = _weights_pack(W_fwd, W_back)
    use_bias = bool(np.any(b_fwd) or np.any(b_back))
    use_gb = bool(np.any(gamma != 1.0) or np.any(beta != 0.0))

    key = (T_dr, use_bias, use_gb)
    if key not in _NC_CACHE:
        _NC_CACHE[key] = _build_nc(T_dr, use_bias, use_gb)
    nc = _NC_CACHE[key]

    in_maps = []
    for c in range(N_CORES):
        m = {
            "stream8": pk["stream8"][c],
            "stream16": pk["stream16"][c],
            "ind8": pk["ind8"][c],
            "ind16": pk["ind16"][c],
            "hres": pk["hres"][c],
            "recip": pk["recip"][c],
            "wf": wf,
            "wb": wb,
        }
        if use_bias:
            recip_all = 1.0 / np.maximum(pk["cnt"], 1).astype(np.float32)
            bcv = (pk["cnt_f"][:, None] * b_fwd[None, :]
                   + pk["cnt_b"][:, None] * b_back[None, :]) \
                * recip_all[:, None]
            ids = pk["node_ids"].reshape(NWIN, 128)
            safe = np.maximum(ids, 0)
            bc = bcv[safe]
            bc[ids < 0] = 0.0
            m["bc"] = np.ascontiguousarray(
                bc.reshape(N_CORES, WPC * 128, D)[c], dtype=np.float32)
        if use_gb:
            m["gam"] = gamma.reshape(1, D)
            m["bet"] = beta.reshape(1, D)
        in_maps.append(m)

    kwargs = {}
    if PROFILE:
        try:
            import antenv.axon_hooks  # noqa: F401
            kwargs = dict(trace=True, trace_cores=[0])
        except ImportError:
            pass
    res = run_bass_kernel_spmd(nc, in_maps, core_ids=list(range(N_CORES)),
                               **kwargs)
    LAST["exec_time_ns"] = res.exec_time_ns
    LAST["results"] = res

    out = np.empty((N_NODES, D), dtype=np.float32)
    ids = pk["node_ids"]  # [NWIN, 128]
    for c in range(N_CORES):
        r = res.results[c]["out"]  # [npr, 128, 2, D]
        rows = np.ascontiguousarray(
            r.transpose(0, 2, 1, 3)).reshape(-1, D).astype(np.float32)
        wids = ids[c * WPC:(c + 1) * WPC].reshape(-1)
        valid = wids >= 0
        out[wids[valid]] = rows[valid]
    return out


# revision 27
# speedup vs baseline: 1.0599x; 1.0599x over previous
"""Trainium2 Bass kernel for a GNN message-passing layer.

Math (matches the reference):
  msg_fwd(e)  = concat(H[head], E[e], H[head]+E[e], H[head]*E[e]) @ W_fwd.T + b_fwd
  msg_back(e) = concat(H[tail], E[e], H[tail]+E[e], H[tail]*E[e]) @ W_back.T + b_back
  agg[v] = mean of messages destined to v   (fwd -> tail, back -> head)
  out = LN(leaky_relu(agg) + H) * gamma + beta

Using linearity of the concat GEMM:
  msg = Hh @ (W1+W3).T + E @ (W2+W3).T + (Hh*E) @ W4.T  (+ bias)
and linearity of the segment-sum, each destination node only needs the three
768-wide raw sums  [sum Hh | sum E | sum Hh*E]  per direction, followed by a
small per-node GEMM with the combined weights.

v2: destinations (nodes) are packed into 128-node windows (50 per core). The
segment-sum is a one-hot scatter matmul; the bulk of the [Hh|E] stream is
shipped in fp8e4 and scattered with DoubleRow fp8 matmuls (2 contraction
rows per column step). Messages whose destination has a small total degree
(the nodes most sensitive to quantization error) are routed through a
full-fp16 tile per window/direction. The Hh*E products are computed on
device (DVE/GpSimd) in fp16 from the shipped operands and scattered with
plain matmuls. The per-node mean (1/cnt) rides the PSUM-drain scale; the
LayerNorm tail is balanced across Scalar/Vector/GpSimd engines.
"""

import os
import numpy as np
import ml_dtypes

import concourse.bass as bass
import concourse.bacc as bacc
import concourse.mybir as mybir
import concourse.tile as tile
from concourse.masks import make_identity
from concourse.bass_utils import run_bass_kernel_spmd

N_NODES = 50000
N_EDGES = 250000
D = 256
LEAKY = 0.01
LN_EPS = 1e-5

N_CORES = 8
WPC = 50                      # windows per core
NWIN = N_CORES * WPC          # 400 windows of <=128 nodes
CTH = 3                       # dst total-degree threshold for fp16 routing
PROFILE = bool(int(os.environ.get("KERNEL_TRACE", "0")))
LAST = {}                     # debug/profiling info from the last call

F32 = mybir.dt.float32
F16 = mybir.dt.float16
F8 = mybir.dt.float8e4
DRMODE = mybir.MatmulPerfMode.DoubleRow
E4M3 = ml_dtypes.float8_e4m3


# ----------------------------------------------------------------- host side

def _pack_nodes(cnt_f, cnt_b, cap):
    """Assign each node to one of NWIN windows (<=128 nodes each) such that
    per-window fwd/back message counts stay <= cap. Greedy min-max."""
    order = np.argsort(-(cnt_f + cnt_b), kind="stable")
    F = np.zeros(NWIN, dtype=np.int64)
    B = np.zeros(NWIN, dtype=np.int64)
    NN = np.zeros(NWIN, dtype=np.int64)
    win_of = np.empty(N_NODES, dtype=np.int64)
    loc_of = np.empty(N_NODES, dtype=np.int64)
    BIG = np.int64(1) << 60
    for v in order:
        cf = cnt_f[v]
        cb = cnt_b[v]
        score = np.maximum(F + cf, B + cb)
        bad = (NN >= 128) | (F + cf > cap) | (B + cb > cap)
        score = np.where(bad, BIG, score)
        w = int(np.argmin(score))
        if score[w] >= BIG:
            return None
        win_of[v] = w
        loc_of[v] = NN[w]
        F[w] += cf
        B[w] += cb
        NN[w] += 1
    return win_of, loc_of, NN


def _pack_host(H, E, ht, T_dr):
    capq = T_dr * 256             # fp8 (DoubleRow) slots per window-dir
    cap = capq + 128              # plus the fp16 tile
    nq = 2 * T_dr                 # fp8 128-row planes per window-dir
    heads = ht[:, 0].astype(np.int64)
    tails = ht[:, 1].astype(np.int64)
    cnt_f = np.bincount(tails, minlength=N_NODES)
    cnt_b = np.bincount(heads, minlength=N_NODES)
    cnt = cnt_f + cnt_b

    packed = _pack_nodes(cnt_f, cnt_b, cap)
    if packed is None:
        return None
    win_of, loc_of, NN = packed

    H8 = H.astype(E4M3)
    E8 = E.astype(E4M3)
    H16 = H.astype(np.float16)
    E16 = E.astype(np.float16)

    # partition-major layouts so each DMA is one contiguous run per partition
    stream8 = np.zeros((NWIN, 2, 128, nq, 2 * D), dtype=E4M3)
    stream16 = np.zeros((NWIN, 2, 128, 2 * D), dtype=np.float16)
    ind8 = np.zeros((NWIN, 2, 128, nq, 128), dtype=E4M3)
    ind16 = np.zeros((NWIN, 2, 128, 128), dtype=E4M3)

    for d, (src, dst) in enumerate(((heads, tails), (tails, heads))):
        w = win_of[dst]
        c = cnt[dst]
        order = np.lexsort((c, w))        # window-major, low total degree first
        ws = w[order]
        counts = np.bincount(ws, minlength=NWIN)
        starts = np.zeros(NWIN + 1, dtype=np.int64)
        np.cumsum(counts, out=starts[1:])
        pos = np.arange(len(ws), dtype=np.int64) - starts[ws]
        # the 128 fp16 slots are always paid for: fill them with the
        # lowest-degree destinations (the quantization-sensitive ones)
        k16 = np.minimum(128, counts)
        is16 = pos < k16[ws]

        e16 = order[is16]
        s16 = pos[is16]
        stream16[ws[is16], d, s16, :D] = H16[src[e16]]
        stream16[ws[is16], d, s16, D:] = E16[e16]
        ind16[ws[is16], d, s16, loc_of[dst[e16]]] = 1.0

        e8 = order[~is16]
        r8 = pos[~is16] - k16[ws[~is16]]
        q8 = r8 >> 7                      # plane index
        p8 = r8 & 127                     # partition
        w8 = ws[~is16]
        stream8[w8, d, p8, q8, :D] = H8[src[e8]]
        stream8[w8, d, p8, q8, D:] = E8[e8]
        ind8[w8, d, p8, q8, loc_of[dst[e8]]] = 1.0

    node_ids = np.full((NWIN, 128), -1, dtype=np.int64)
    node_ids[win_of, loc_of] = np.arange(N_NODES, dtype=np.int64)

    recip_all = 1.0 / np.maximum(cnt, 1).astype(np.float32)
    safe_ids = np.maximum(node_ids, 0)
    hres = H16[safe_ids]                     # [NWIN, 128, D]
    hres[node_ids < 0] = 0.0
    recip = recip_all[safe_ids].astype(np.float32)
    recip[node_ids < 0] = 1.0

    # batch pairs of windows into single contiguous-per-partition DMA images
    npr = WPC // 2
    s8 = stream8.reshape(N_CORES, npr, 2, 2, 128, nq, 2 * D)
    s8 = np.ascontiguousarray(s8.transpose(0, 1, 4, 2, 3, 5, 6)).reshape(
        N_CORES, npr, 128, 4 * nq, 2 * D)
    s16 = stream16.reshape(N_CORES, npr, 2, 2, 128, 2 * D)
    s16 = np.ascontiguousarray(s16.transpose(0, 1, 4, 2, 3, 5)).reshape(
        N_CORES, npr, 128, 4, 2 * D)
    i8 = ind8.reshape(N_CORES, npr, 2, 2, 128, nq, 128)
    i8 = np.ascontiguousarray(i8.transpose(0, 1, 4, 2, 3, 5, 6)).reshape(
        N_CORES, npr, 128, 4 * nq, 128)
    i16 = ind16.reshape(N_CORES, npr, 2, 2, 128, 128)
    i16 = np.ascontiguousarray(i16.transpose(0, 1, 4, 2, 3, 5)).reshape(
        N_CORES, npr, 128, 4, 128)
    hr = hres.reshape(N_CORES, npr, 2, 128, D)
    hr = np.ascontiguousarray(hr.transpose(0, 1, 3, 2, 4))
    return {
        "stream8": s8,
        "stream16": s16,
        "ind8": i8,
        "ind16": i16,
        "hres": hr,
        "recip": recip.reshape(N_CORES, WPC, 128).transpose(0, 2, 1).copy(),
        "node_ids": node_ids,
        "cnt_f": cnt_f,
        "cnt_b": cnt_b,
        "cnt": cnt,
    }


def _weights_pack(W_fwd, W_back):
    def cat(W):
        W1, W2, W3, W4 = (W[:, i * D:(i + 1) * D] for i in range(4))
        return np.concatenate([(W1 + W3).T, (W2 + W3).T, W4.T], axis=0)

    wf = np.ascontiguousarray(cat(W_fwd).reshape(6, 128, D), dtype=np.float16)
    wb6 = cat(W_back).reshape(6, 128, D)
    # acc layout: blocks 0..3 f[Hh|E], 4..5 f[HE], 6..7 b[HE], 8..11 b[Hh|E]
    wb = np.ascontiguousarray(wb6[[4, 5, 0, 1, 2, 3]], dtype=np.float16)
    return wf, wb


# --------------------------------------------------------------- device side

def _build_nc(T_dr, use_bias, use_gb):
    nc = bacc.Bacc()
    nq = 2 * T_dr
    npr = WPC // 2

    st8_d = nc.dram_tensor("stream8", [npr, 128, 4 * nq, 2 * D], F8,
                           kind="ExternalInput")
    st16_d = nc.dram_tensor("stream16", [npr, 128, 4, 2 * D], F16,
                            kind="ExternalInput")
    ind8_d = nc.dram_tensor("ind8", [npr, 128, 4 * nq, 128], F8,
                            kind="ExternalInput")
    ind16_d = nc.dram_tensor("ind16", [npr, 128, 4, 128], F8,
                             kind="ExternalInput")
    hres_d = nc.dram_tensor("hres", [npr, 128, 2, D], F16,
                            kind="ExternalInput")
    recip_d = nc.dram_tensor("recip", [128, WPC], F32, kind="ExternalInput")
    wf_d = nc.dram_tensor("wf", [6, 128, D], F16, kind="ExternalInput")
    wb_d = nc.dram_tensor("wb", [6, 128, D], F16, kind="ExternalInput")
    if use_bias:
        bc_d = nc.dram_tensor("bc", [WPC * 128, D], F32, kind="ExternalInput")
    if use_gb:
        gam_d = nc.dram_tensor("gam", [1, D], F32, kind="ExternalInput")
        bet_d = nc.dram_tensor("bet", [1, D], F32, kind="ExternalInput")
    out_d = nc.dram_tensor("out", [npr, 128, 2, D], F16,
                           kind="ExternalOutput")

    with tile.TileContext(nc) as tc:
        with (
            tc.tile_pool(name="const", bufs=1) as constp,
            tc.tile_pool(name="stream", bufs=3) as streamp,
            tc.tile_pool(name="he", bufs=8) as hep,
            tc.tile_pool(name="ind", bufs=3) as indp,
            tc.tile_pool(name="aggsb", bufs=3) as aggsbp,
            tc.tile_pool(name="aggT", bufs=3) as aggTp,
            tc.tile_pool(name="tailp", bufs=3) as tailp,
            tc.tile_pool(name="outp", bufs=4) as outp,
            tc.tile_pool(name="pacc", bufs=2, space="PSUM") as pacc,
            tc.tile_pool(name="pmisc", bufs=1, space="PSUM") as pmisc,
        ):
            ident32 = constp.tile([128, 128], F32)
            make_identity(nc, ident32)
            ident = constp.tile([128, 128], F16)
            nc.vector.tensor_copy(out=ident, in_=ident32)
            wf_sb = constp.tile([128, 6, D], F16)
            nc.sync.dma_start(out=wf_sb,
                              in_=wf_d[:, :, :].rearrange("c k n -> k c n"))
            wb_sb = constp.tile([128, 6, D], F16)
            nc.sync.dma_start(out=wb_sb,
                              in_=wb_d[:, :, :].rearrange("c k n -> k c n"))
            recip_sb = constp.tile([128, WPC], F32)
            nc.sync.dma_start(out=recip_sb, in_=recip_d[:, :])
            eps_sb = constp.tile([128, 1], F32)
            nc.vector.memset(eps_sb, LN_EPS)
            if use_gb:
                gam_sb = constp.tile([128, D], F32)
                nc.sync.dma_start(
                    out=gam_sb,
                    in_=bass.AP(tensor=gam_d, offset=0, ap=[[0, 128], [1, D]]),
                )
                bet_sb = constp.tile([128, D], F32)
                nc.sync.dma_start(
                    out=bet_sb,
                    in_=bass.AP(tensor=bet_d, offset=0, ap=[[0, 128], [1, D]]),
                )

            def node_side(w, acc, hresp, op):
                """Drain, transpose, per-node GEMM and LayerNorm tail for
                window w (emitted one window behind the scatter so the
                drain/transpose ping-pong hides behind scatter matmuls)."""
                wlo = w & 1
                # drain PSUM to fp16 in halves (raw sums; 1/cnt at prelu)
                aggsb = aggsbp.tile([128, 1536], F16)
                nc.scalar.copy(out=aggsb[:, 0:768], in_=acc[:, 0:768])
                nc.scalar.copy(out=aggsb[:, 768:1536], in_=acc[:, 768:1536])

                aggTh = []
                for h in range(2):
                    tp = pmisc.tile([128, 768], F16, tag="tp")
                    for j in range(6):
                        blk = h * 6 + j
                        nc.tensor.transpose(
                            tp[:, j * 128:(j + 1) * 128],
                            aggsb[:, blk * 128:(blk + 1) * 128], ident)
                    aggT = aggTp.tile([128, 6, 128], F16, tag=f"t{h}")
                    if h == 0:
                        nc.vector.tensor_copy(out=aggT, in_=tp)
                    else:
                        nc.scalar.copy(out=aggT, in_=tp)
                    aggTh.append(aggT)

                nodeps = pmisc.tile([128, D], F32, tag="nd")
                for blk in range(12):
                    rhs = (wf_sb[:, blk, :] if blk < 6
                           else wb_sb[:, blk - 6, :])
                    nc.tensor.matmul(
                        nodeps, aggTh[blk // 6][:, blk % 6, :], rhs,
                        start=(blk == 0), stop=(blk == 11))

                x = tailp.tile([128, D], F32, tag="x")
                if use_bias:
                    y = tailp.tile([128, D], F32, tag="y")
                    nc.scalar.activation(
                        out=y, in_=nodeps,
                        func=mybir.ActivationFunctionType.Copy,
                        bias=0.0, scale=recip_sb[:, w:w + 1])
                    bc_sb = tailp.tile([128, D], F32, tag="bc")
                    nc.sync.dma_start(
                        out=bc_sb, in_=bc_d[w * 128:(w + 1) * 128, :])
                    nc.gpsimd.tensor_add(y, y, bc_sb)
                    nc.scalar.activation(
                        out=x, in_=y,
                        func=mybir.ActivationFunctionType.Prelu,
                        bias=0.0, scale=1.0, alpha=LEAKY)
                else:
                    nc.scalar.activation(
                        out=x, in_=nodeps,
                        func=mybir.ActivationFunctionType.Prelu,
                        bias=0.0, scale=recip_sb[:, w:w + 1], alpha=LEAKY)

                nc.gpsimd.tensor_add(x, x, hresp[:, wlo, :])

                stats = tailp.tile([128, 6], F32, tag="stats")
                nc.vector.bn_stats(out=stats, in_=x)
                mv = tailp.tile([128, 2], F32, tag="mv")
                nc.vector.bn_aggr(out=mv, in_=stats)
                std = tailp.tile([128, 1], F32, tag="std")
                nc.scalar.activation(
                    out=std, in_=mv[:, 1:2],
                    func=mybir.ActivationFunctionType.Sqrt,
                    bias=eps_sb, scale=1.0)
                rstd = tailp.tile([128, 1], F32, tag="rstd")
                nc.vector.reciprocal(out=rstd, in_=std)
                nmr = tailp.tile([128, 1], F32, tag="nmr")
                nc.vector.tensor_scalar(
                    out=nmr, in0=mv[:, 0:1], scalar1=rstd, scalar2=-1.0,
                    op0=mybir.AluOpType.mult, op1=mybir.AluOpType.mult)

                nc.scalar.activation(
                    out=op[:, wlo, :], in_=x,
                    func=mybir.ActivationFunctionType.Identity,
                    bias=nmr, scale=rstd)
                if use_gb:
                    nc.vector.tensor_tensor(
                        out=op[:, wlo, :], in0=op[:, wlo, :], in1=gam_sb,
                        op=mybir.AluOpType.mult)
                    nc.vector.tensor_tensor(
                        out=op[:, wlo, :], in0=op[:, wlo, :], in1=bet_sb,
                        op=mybir.AluOpType.add)
                if wlo == 1:
                    # store from the gpsimd queue: the sync queue is in-order
                    # and a store waiting on compute would block the next
                    # pair's input prefetch
                    nc.gpsimd.dma_start(out=out_d[w // 2], in_=op)

            pending = None  # (w, acc, hresp, op) awaiting node_side
            for pr in range(npr):
                st8p = streamp.tile([128, 4 * nq, 2 * D], F8, tag="st8")
                nc.sync.dma_start(out=st8p, in_=st8_d[pr])
                st16p = streamp.tile([128, 4, 2 * D], F16, tag="st16")
                nc.sync.dma_start(out=st16p, in_=st16_d[pr])
                ind8p = indp.tile([128, 4 * nq, 128], F8, tag="i8")
                nc.sync.dma_start(out=ind8p, in_=ind8_d[pr])
                ind16p = indp.tile([128, 4, 128], F8, tag="i16")
                nc.sync.dma_start(out=ind16p, in_=ind16_d[pr])
                hresp = tailp.tile([128, 2, D], F16, tag="hres")
                nc.sync.dma_start(out=hresp, in_=hres_d[pr])
                op = outp.tile([128, 2, D], F16)

                # products for all four window-dir groups up front
                he8s = []
                he16s = []
                for g in range(4):
                    base = g * nq
                    he8 = hep.tile([128, nq, D], F8, tag="he8")
                    nc.vector.tensor_tensor(
                        out=he8, in0=st8p[:, base:base + nq, 0:D],
                        in1=st8p[:, base:base + nq, D:2 * D],
                        op=mybir.AluOpType.mult)
                    he8s.append(he8)
                    he16 = hep.tile([128, D], F16, tag="he16")
                    nc.gpsimd.tensor_tensor(
                        out=he16, in0=st16p[:, g, 0:D],
                        in1=st16p[:, g, D:2 * D],
                        op=mybir.AluOpType.mult)
                    he16s.append(he16)

                for wlo in range(2):
                    w = pr * 2 + wlo
                    # acc columns (fp32, 3 PSUM banks):
                    #   0:512    f [sumHh | sumE]
                    #   512:768  f [sumHE]
                    #   768:1024 b [sumHE]
                    #   1024:1536 b [sumHh | sumE]
                    acc = pacc.tile([128, 1536], F32)
                    # [Hh|E] for both directions first (only needs the DMAs),
                    # then the HE matmuls (need the DVE/GpSimd products)
                    for d in range(2):
                        g = wlo * 2 + d
                        base = g * nq
                        cs = (0, 512) if d == 0 else (1024, 1536)
                        for t in range(T_dr):
                            nc.tensor.matmul(
                                acc[:, cs[0]:cs[1]],
                                ind8p[:, base + 2 * t:base + 2 * t + 2, :],
                                st8p[:, base + 2 * t:base + 2 * t + 2, :],
                                start=(t == 0), stop=False, perf_mode=DRMODE)
                        nc.tensor.matmul(
                            acc[:, cs[0]:cs[1]], ind16p[:, g, :],
                            st16p[:, g, :], start=False, stop=True)
                    for d in range(2):
                        g = wlo * 2 + d
                        base = g * nq
                        ch = (512, 768) if d == 0 else (768, 1024)
                        for t in range(T_dr):
                            nc.tensor.matmul(
                                acc[:, ch[0]:ch[1]],
                                ind8p[:, base + 2 * t:base + 2 * t + 2, :],
                                he8s[g][:, 2 * t:2 * t + 2, :],
                                start=(t == 0), stop=False, perf_mode=DRMODE)
                        nc.tensor.matmul(
                            acc[:, ch[0]:ch[1]], ind16p[:, g, :], he16s[g],
                            start=False, stop=True)

                    if pending is not None:
                        node_side(*pending)
                    pending = (w, acc, hresp, op)
            node_side(*pending)

    nc.compile()
    return nc


_NC_CACHE = {}


def kernel(H, E, ht, W_fwd, b_fwd, W_back, b_back, gamma, beta):
    H = np.asarray(H, dtype=np.float32)
    E = np.asarray(E, dtype=np.float32)
    ht = np.asarray(ht)
    W_fwd = np.asarray(W_fwd, dtype=np.float32)
    W_back = np.asarray(W_back, dtype=np.float32)
    b_fwd = np.asarray(b_fwd, dtype=np.float32)
    b_back = np.asarray(b_back, dtype=np.float32)
    gamma = np.asarray(gamma, dtype=np.float32)
    beta = np.asarray(beta, dtype=np.float32)

    T_dr = 2
    pk = _pack_host(H, E, ht, T_dr)
    if pk is None:
        T_dr = 3
        pk = _pack_host(H, E, ht, T_dr)
        assert pk is not None, "window packing failed even at T_dr=3"

    wf, wb = _weights_pack(W_fwd, W_back)
    use_bias = bool(np.any(b_fwd) or np.any(b_back))
    use_gb = bool(np.any(gamma != 1.0) or np.any(beta != 0.0))

    key = (T_dr, use_bias, use_gb)
    if key not in _NC_CACHE:
        _NC_CACHE[key] = _build_nc(T_dr, use_bias, use_gb)
    nc = _NC_CACHE[key]

    in_maps = []
    for c in range(N_CORES):
        m = {
            "stream8": pk["stream8"][c],
            "stream16": pk["stream16"][c],
            "ind8": pk["ind8"][c],
            "ind16": pk["ind16"][c],
            "hres": pk["hres"][c],
            "recip": pk["recip"][c],
            "wf": wf,
            "wb": wb,
        }
        if use_bias:
            recip_all = 1.0 / np.maximum(pk["cnt"], 1).astype(np.float32)
            bcv = (pk["cnt_f"][:, None] * b_fwd[None, :]
                   + pk["cnt_b"][:, None] * b_back[None, :]) \
                * recip_all[:, None]
            ids = pk["node_ids"].reshape(NWIN, 128)
            safe = np.maximum(ids, 0)
            bc = bcv[safe]
            bc[ids < 0] = 0.0
            m["bc"] = np.ascontiguousarray(
                bc.reshape(N_CORES, WPC * 128, D)[c], dtype=np.float32)
        if use_gb:
            m["gam"] = gamma.reshape(1, D)
            m["bet"] = beta.reshape(1, D)
        in_maps.append(m)

    kwargs = {}
    if PROFILE:
        try:
            import antenv.axon_hooks  # noqa: F401
            kwargs = dict(trace=True, trace_cores=[0])
        except ImportError:
            pass
    res = run_bass_kernel_spmd(nc, in_maps, core_ids=list(range(N_CORES)),
                               **kwargs)
    LAST["exec_time_ns"] = res.exec_time_ns
    LAST["results"] = res

    out = np.empty((N_NODES, D), dtype=np.float32)
    ids = pk["node_ids"]  # [NWIN, 128]
    for c in range(N_CORES):
        r = res.results[c]["out"]  # [npr, 128, 2, D]
        rows = np.ascontiguousarray(
            r.transpose(0, 2, 1, 3)).reshape(-1, D).astype(np.float32)
        wids = ids[c * WPC:(c + 1) * WPC].reshape(-1)
        valid = wids >= 0
        out[wids[valid]] = rows[valid]
    return out
